# revision 11
# baseline (speedup 1.0000x reference)
"""Locoformer on 8 Trainium2 NeuronCores.

Sharding: 8-way sequence parallel. B*S = 2*2048 = 4096 tokens -> 8 chunks of
512 tokens (core c: batch c//4, seq chunk c%4). Each core runs the full
4-layer model on its 512 tokens. The sliding-window (512) attention needs a
512-token k/v halo from the left neighbor; exchanged per layer via a bf16
AllGather of (rope'd k, mixed v) with a 9-slot receive buffer (slot pid reads
rank pid-1; core 0/4's halo is garbage but masked out via key-validity bias).

Layouts: activations live feature-on-partition ("T layout", [128, chunk,
tok]); matmuls chain without transposes except q/k/o (PE transposes).
Weights host-cast to bf16, fp32 residual stream, fp32 softmax/norm stats.
"""

import sys

import numpy as np

sys.path.insert(0, "/opt/trn_rl_repo")

import ml_dtypes
import concourse.bass as bass
import concourse.mybir as mybir
import concourse.tile as tile
from concourse import bacc
from concourse.bass import ds
from concourse.bass_utils import run_bass_kernel_spmd
from concourse.masks import make_identity

F32 = mybir.dt.float32
BF16 = mybir.dt.bfloat16
AF = mybir.ActivationFunctionType

B, S, DIM, H, DH, L, WIN = 2, 2048, 1024, 16, 64, 4, 512
DIN = 2730
DINP = 2816  # padded to 22*128
HC = DINP // 128  # 22 hidden chunks
FC = DIM // 128  # 8 feature chunks
TOK = 512  # tokens per core
TT = TOK // 128  # 4 token tiles
KEYS = 1024  # halo 512 + own 512
KC = KEYS // 128
EPS = 1.1920929e-07
SCALE = DH ** -0.5
NEG = -1e30
N_CORES = 8

KT_SZ = DIM * TOK  # kT region elems (per hp block of 128x512)
V_OFF = KT_SZ  # v region offset in kv block
KVBLK = KT_SZ + TOK * DIM  # 1 MiB elems bf16 = 2MB


def bcast_free(ap, n, pos):
    """Insert a step-0 free dim of size n at position pos (after partition)."""
    aps = [list(p) for p in ap.ap]
    aps.insert(pos, [0, n])
    return bass.AP(tensor=ap.tensor, offset=ap.offset, ap=aps)


def strided65(ap):
    """Reinterpret a [128, 1040] v_aug chunk slice as [128, 16, 64] skipping
    the ones column at 64 of each 65-block."""
    return bass.AP(
        tensor=ap.tensor, offset=ap.offset, ap=[list(ap.ap[0]), [65, 16], [1, 64]]
    )


def ones_cols(ap):
    """The 16 ones-columns (index 64 of each 65-block) of a v_aug chunk."""
    return bass.AP(
        tensor=ap.tensor, offset=ap.offset + 64, ap=[list(ap.ap[0]), [65, 16]]
    )


def eo_ap(ap, half):
    """Even/odd half-blocks of a [128, 1024] q/k tile: per head 64-col block,
    cols [0:32) (half=0) or [32:64) (half=1) -> [128, 16, 32]."""
    return bass.AP(
        tensor=ap.tensor,
        offset=ap.offset + 32 * half,
        ap=[list(ap.ap[0]), [64, 16], [1, 32]],
    )


def build_nc(single=False):
    nc = bacc.Bacc("TRN2", num_devices=1 if single else N_CORES)

    # ---- dram I/O ----
    xT0 = nc.dram_tensor("xT0", [DIM, TOK], F32, kind="ExternalInput")
    wq = nc.dram_tensor("wq", [L, DIM, DIM], BF16, kind="ExternalInput")
    wk = nc.dram_tensor("wk", [L, DIM, DIM], BF16, kind="ExternalInput")
    wv = nc.dram_tensor("wv", [L, DIM, DIM], BF16, kind="ExternalInput")
    wgm = nc.dram_tensor("wgm", [L, DIM, 32], BF16, kind="ExternalInput")
    # wo rearranged: [L, FC(dim-chunk mc), HD, 128]
    wo = nc.dram_tensor("wo", [L, FC, DIM, 128], BF16, kind="ExternalInput")
    # w1 rearranged: [L, 2*HC(hid chunk j: a at j, g at HC+j), DIM, 128]
    w1 = nc.dram_tensor("w1", [L, 2 * HC, DIM, 128], BF16, kind="ExternalInput")
    # w2 rearranged: [L, FC, DINP, 128]
    w2 = nc.dram_tensor("w2", [L, FC, DINP, 128], BF16, kind="ExternalInput")
    b1a = nc.dram_tensor("b1a", [L, DINP], F32, kind="ExternalInput")
    b1g = nc.dram_tensor("b1g", [L, DINP], F32, kind="ExternalInput")
    b2 = nc.dram_tensor("b2", [L, DIM], F32, kind="ExternalInput")
    cos_in = nc.dram_tensor("cos_in", [TOK, 32], BF16, kind="ExternalInput")
    sin_in = nc.dram_tensor("sin_in", [TOK, 32], BF16, kind="ExternalInput")
    keyvalid = nc.dram_tensor("keyvalid", [KEYS], F32, kind="ExternalInput")
    outT = nc.dram_tensor("outT", [DIM, TOK], F32, kind="ExternalOutput")

    with tile.TileContext(nc) as tc:
        import contextlib

        stack = contextlib.ExitStack()
        with stack:
            persist = stack.enter_context(tc.tile_pool(name="persist", bufs=1))
            wpool = stack.enter_context(tc.tile_pool(name="wpool", bufs=2))
            w1pool = stack.enter_context(tc.tile_pool(name="w1pool", bufs=2))
            w2pool = stack.enter_context(tc.tile_pool(name="w2pool", bufs=1))
            wopool = stack.enter_context(tc.tile_pool(name="wopool", bufs=1))
            scratch = stack.enter_context(tc.tile_pool(name="scratch", bufs=2))
            scratch2 = stack.enter_context(tc.tile_pool(name="scratch2", bufs=1))
            pbuf = stack.enter_context(tc.tile_pool(name="pbuf", bufs=2))
            small = stack.enter_context(tc.tile_pool(name="small", bufs=2))
            rowpool = stack.enter_context(tc.tile_pool(name="rowpool", bufs=1))
            dram = stack.enter_context(tc.tile_pool(name="dram", bufs=1, space="DRAM"))

            pid = nc.gpsimd.partition_id()

            # ---- persistent state ----
            xT = persist.tile([128, FC, TOK], F32)  # residual stream (T)
            xbf = persist.tile([128, FC, TOK], BF16)  # bf16 mirror
            kT = persist.tile([128, FC, KEYS], BF16)  # [2-head d, keys]
            qT = persist.tile([128, FC, TOK], BF16)
            v_aug = persist.tile([128, KC, 16 * 65], BF16)  # [key, h*65]
            vres = persist.tile([128, TT, DIM], BF16)  # layer-0 v (natural)
            qkv_nat = persist.tile([128, TT, 3, DIM], BF16)  # q|k|v natural
            hidT = persist.tile([128, HC, TOK], BF16)
            fT = persist.tile([128, FC, TOK], BF16)
            gm_t = persist.tile([128, TT, 32], F32)  # gates | mix (natural)
            rs_q = persist.tile([128, TT, 1], F32)  # rs*scale for q
            rs_a = persist.tile([128, TT, 1], F32)  # rs for k/v/gm
            cos_t = persist.tile([128, TT, 32], BF16)
            sin_t = persist.tile([128, TT, 32], BF16)
            kv_t = persist.tile([128, KC, 1], F32)  # keyvalid bias
            m_diag = persist.tile([128, 128], F32)
            m_far = persist.tile([128, 128], F32)
            ident = persist.tile([128, 128], BF16)
            ones_bf = persist.tile([128, 1], BF16)
            one_f = persist.tile([1, 1], F32)
            rsb = persist.tile([128, TOK], F32)  # broadcast norm scale
            eps_t = persist.tile([128, 1], F32)
            b1a_all = persist.tile([128, HC], F32)
            b1g_all = persist.tile([128, HC], F32)
            b2_all = persist.tile([128, FC], F32)
            eps1 = persist.tile([1, 1], F32)

            kv_in = dram.tile([KVBLK], BF16)
            kv_out9 = dram.tile([9 * KVBLK], BF16)

            # ---- prologue ----
            for kc in range(FC):
                nc.sync.dma_start(xT[:, kc, :], xT0[128 * kc : 128 * (kc + 1), :])
                nc.vector.tensor_copy(xbf[:, kc, :], xT[:, kc, :])
            for tq in range(TT):
                nc.sync.dma_start(cos_t[:, tq, :], cos_in[128 * tq : 128 * (tq + 1), :])
                nc.sync.dma_start(sin_t[:, tq, :], sin_in[128 * tq : 128 * (tq + 1), :])
            for kc in range(KC):
                nc.sync.dma_start(
                    kv_t[:, kc, :],
                    keyvalid[128 * kc : 128 * (kc + 1)].rearrange("(p o) -> p o", p=128),
                )
            nc.vector.memset(eps_t[:], EPS)
            nc.vector.memset(eps1[:], EPS)
            nc.vector.memset(ones_bf[:], 1.0)
            nc.vector.memset(one_f[:], 1.0)
            make_identity(nc, ident[:])
            # additive band masks in simT layout [key p, tok f]:
            # diag block (kc==tq): valid iff p >= f ; far block (kc==tq+4): p <= f
            nc.gpsimd.memset(m_diag[:], 0.0)
            nc.gpsimd.affine_select(
                out=m_diag[:], in_=m_diag[:], compare_op=mybir.AluOpType.is_ge,
                fill=NEG, base=0, pattern=[[-1, 128]], channel_multiplier=1,
            )
            nc.gpsimd.memset(m_far[:], 0.0)
            nc.gpsimd.affine_select(
                out=m_far[:], in_=m_far[:], compare_op=mybir.AluOpType.is_ge,
                fill=NEG, base=0, pattern=[[1, 128]], channel_multiplier=-1,
            )
            # ones columns of v_aug (persist across layers; v writes skip them)
            for kc in range(KC):
                nc.vector.memset(ones_cols(v_aug[:, kc, :]), 1.0)
            # zero slot 0 of kv_out9 so core 0's (masked) halo reads finite data
            zt = scratch2.tile([128, 1024], BF16, tag="onetime")
            nc.vector.memset(zt[:], 0.0)
            for i in range(8):
                nc.gpsimd.dma_start(
                    kv_out9[i * 131072 : (i + 1) * 131072].rearrange(
                        "(p f) -> p f", p=128
                    ),
                    zt[:],
                )

            def norm_stats(psum_pool, name):
                """sum over features of xT^2 -> psum [1, TOK] (fp32)."""
                ssq = psum_pool.tile([1, TOK], F32, tag=f"ssq{name}")
                for kc in range(FC):
                    sq = scratch2.tile([128, TOK], BF16, tag="sq")
                    nc.vector.tensor_mul(sq[:], xT[:, kc, :], xT[:, kc, :])
                    nc.tensor.matmul(
                        ssq[:], ones_bf[:], sq[:],
                        start=(kc == 0), stop=(kc == FC - 1),
                    )
                ssq_sb = rowpool.tile([1, TOK], F32, tag="v1")
                nc.vector.tensor_copy(ssq_sb[:], ssq[:])
                return ssq_sb

            def rsqrt_act(dst, src_ap, eps_ap, lnv, scale=1.0):
                """dst = (src*scale + EPS)^-0.5 via exp(-0.5*ln(.))."""
                nc.scalar.activation(lnv, src_ap, AF.Ln, bias=eps_ap, scale=scale)
                nc.scalar.activation(dst, lnv, AF.Exp, scale=-0.5)

            # ================= layers =================
            for l in range(L):
                # ---- attn norm scale, transposed to per-token partitions ----
                with tc.tile_pool(name=f"ps_n1_{l}", bufs=2, space="PSUM") as pp:
                    ssq_sb = norm_stats(pp, f"n1_{l}")
                    for tq in range(TT):
                        st = pp.tile([128, 1], F32, tag="stat_t")
                        nc.tensor.matmul(
                            st[:], ssq_sb[0:1, 128 * tq : 128 * (tq + 1)], one_f[:],
                            start=True, stop=True,
                        )
                        lnv128 = small.tile([128, 1], F32, tag="lnv128")
                        rsqrt_act(rs_a[:, tq, :], st[:], eps_t[:], lnv128[:], scale=1.0 / DIM)
                        nc.vector.tensor_scalar_mul(
                            rs_q[:, tq, :], rs_a[:, tq, :], SCALE
                        )

                # ---- projections q/k/v/gm per token tile ----
                with tc.tile_pool(name=f"ps_proj_{l}", bufs=4, space="PSUM") as pp, \
                     tc.tile_pool(name=f"ps_gm_{l}", bufs=1, space="PSUM") as ppg, \
                     tc.tile_pool(name=f"ps_tp_{l}", bufs=2, space="PSUM") as ppt:
                    # weight-type-outer streaming: alloc->use->next keeps
                    # the pool trace processable (no forward-release waits)
                    for wi, (wname, wt) in enumerate(
                        (("q", wq), ("k", wk), ("v", wv))
                    ):
                        for nb in range(2):
                            slab = wpool.tile([128, FC, 512], BF16, tag="wproj")
                            nc.sync.dma_start(
                                slab[:],
                                wt[l, :, 512 * nb : 512 * (nb + 1)].rearrange(
                                    "(kc p) n -> p kc n", p=128
                                ),
                            )
                            for tq in range(TT):
                                pt = pp.tile([128, 512], F32, tag="proj")
                                for kc in range(FC):
                                    nc.tensor.matmul(
                                        pt[:],
                                        xbf[:, kc, 128 * tq : 128 * (tq + 1)],
                                        slab[:, kc, :],
                                        start=(kc == 0), stop=(kc == FC - 1),
                                    )
                                rs = rs_q if wname == "q" else rs_a
                                nc.scalar.activation(
                                    qkv_nat[:, tq, wi, 512 * nb : 512 * (nb + 1)],
                                    pt[:], AF.Copy, scale=rs[:, tq, :],
                                )
                    gm_slab = wpool.tile([128, FC, 32], BF16, tag="wgm")
                    nc.sync.dma_start(
                        gm_slab[:], wgm[l].rearrange("(kc p) n -> p kc n", p=128)
                    )

                    for tq in range(TT):
                        qn = qkv_nat[:, tq, 0, :]
                        kn = qkv_nat[:, tq, 1, :]
                        vn = qkv_nat[:, tq, 2, :]
                        # gates/mix: sigmoid(y) = 1/(1+exp(-y))
                        pt = ppg.tile([128, 32], F32, tag="gm")
                        for kc in range(FC):
                            nc.tensor.matmul(
                                pt[:], xbf[:, kc, 128 * tq : 128 * (tq + 1)],
                                gm_slab[:, kc, :],
                                start=(kc == 0), stop=(kc == FC - 1),
                            )
                        negrs = small.tile([128, 1], F32, tag="negrs")
                        nc.vector.tensor_scalar_mul(negrs[:], rs_a[:, tq, :], -1.0)
                        eneg = small.tile([128, 32], F32, tag="eneg")
                        nc.scalar.activation(eneg[:], pt[:], AF.Exp, scale=negrs[:])
                        nc.vector.tensor_scalar_add(eneg[:], eneg[:], 1.0)
                        nc.vector.reciprocal(gm_t[:, tq, :], eneg[:])

                        # rope on q and k (E/O half-blocks, cos/sin bcast)
                        cb = bcast_free(cos_t[:, tq, :], 16, 1)
                        sb_ = bcast_free(sin_t[:, tq, :], 16, 1)
                        for t in (qn, kn):
                            tmpE = scratch.tile([128, 16, 32], BF16, tag="ropeE")
                            tmpO = scratch.tile([128, 16, 32], BF16, tag="ropeO")
                            E, O = eo_ap(t, 0), eo_ap(t, 1)
                            nc.vector.tensor_mul(tmpO[:], O, sb_)  # x_o*sin
                            nc.vector.tensor_mul(tmpE[:], E, sb_)  # x_e*sin
                            nc.vector.tensor_mul(E, E, cb)  # x_e*cos
                            nc.vector.tensor_mul(O, O, cb)  # x_o*cos
                            nc.vector.tensor_sub(E, E, tmpO[:])
                            nc.vector.tensor_add(O, O, tmpE[:])

                        # value residual lerp + write into v_aug (own keys)
                        vdst = strided65(v_aug[:, TT + tq, :])
                        if l == 0:
                            nc.vector.tensor_copy(vres[:, tq, :], vn)
                            nc.vector.tensor_copy(vdst, vn)
                        else:
                            d_ = scratch.tile([128, DIM], BF16, tag="lerp_d")
                            nc.vector.tensor_sub(d_[:], vres[:, tq, :], vn)
                            mixb = bass.AP(
                                tensor=gm_t.tensor,
                                offset=gm_t[:, tq, :].offset + 16,
                                ap=[list(gm_t[:, tq, :].ap[0]), [1, 16], [0, 64]],
                            )
                            dv = d_[:].rearrange("p (h d) -> p h d", h=16)
                            nc.vector.tensor_mul(dv, dv, mixb)
                            nc.vector.tensor_add(
                                vdst, vn.rearrange("p (h d) -> p h d", h=16), dv
                            )

                        # transpose q,k -> qT, kT(own half)
                        for hp in range(FC):
                            tp = ppt.tile([128, 128], BF16, tag="tp")
                            nc.tensor.transpose(
                                tp[:], qn[:, 128 * hp : 128 * (hp + 1)], ident[:]
                            )
                            nc.vector.tensor_copy(
                                qT[:, hp, 128 * tq : 128 * (tq + 1)], tp[:]
                            )
                            tp2 = ppt.tile([128, 128], BF16, tag="tp")
                            nc.tensor.transpose(
                                tp2[:], kn[:, 128 * hp : 128 * (hp + 1)], ident[:]
                            )
                            nc.vector.tensor_copy(
                                kT[:, hp, 512 + 128 * tq : 512 + 128 * (tq + 1)], tp2[:]
                            )

                # ---- kv exchange: send own k/v, AllGather, read halo ----
                for hp in range(FC):
                    nc.sync.dma_start(
                        kv_in[hp * 65536 : (hp + 1) * 65536].rearrange(
                            "(p f) -> p f", p=128
                        ),
                        kT[:, hp, 512:1024],
                    )
                for tq in range(TT):
                    nc.sync.dma_start(
                        kv_in[V_OFF + tq * 131072 : V_OFF + (tq + 1) * 131072].rearrange(
                            "(p h d) -> p h d", p=128, h=16
                        ),
                        strided65(v_aug[:, TT + tq, :]),
                    )
                if single:
                    # timing proxy for the AllGather: move one slot's bytes
                    nc.gpsimd.dma_start(
                        kv_out9[KVBLK : 2 * KVBLK].rearrange("(p f) -> p f", p=128),
                        kv_in[:].rearrange("(p f) -> p f", p=128),
                    )
                else:
                    nc.gpsimd.collective_compute(
                        "AllGather",
                        mybir.AluOpType.bypass,
                        replica_groups=[list(range(N_CORES))],
                        ins=[kv_in[:]],
                        outs=[kv_out9[KVBLK : 9 * KVBLK]],
                    )
                koff = pid * KVBLK
                for hp in range(FC):
                    nc.gpsimd.dma_start(
                        kT[:, hp, 0:512],
                        kv_out9[ds(koff + hp * 65536, 65536)].rearrange(
                            "(p f) -> p f", p=128
                        ),
                    )
                for kc in range(TT):
                    nc.gpsimd.dma_start(
                        strided65(v_aug[:, kc, :]),
                        kv_out9[
                            ds(koff + V_OFF + kc * 131072, 131072)
                        ].rearrange("(p h d) -> p h d", p=128, h=16),
                    )

                # ---- attention ----
                with tc.tile_pool(name=f"ps_att_{l}", bufs=3, space="PSUM") as pa, \
                     tc.tile_pool(name=f"po_att_{l}", bufs=4, space="PSUM") as po:
                    for h in range(H):
                        hp, ho = h // 2, (h % 2) * 64
                        p_sb = pbuf.tile([128, KC, 512], BF16, tag="p_sb")
                        # own keys first (kc>=4) so AG latency overlaps
                        for kc in [4, 5, 6, 7, 0, 1, 2, 3]:
                            qlo = max(0, kc - 4) * 128
                            qhi = min(TT, kc + 1) * 128
                            w = qhi - qlo
                            st = pa.tile([128, 512], F32, tag="sim")
                            nc.tensor.matmul(
                                st[:, 0:w],
                                kT[ho : ho + 64, hp, 128 * kc : 128 * (kc + 1)],
                                qT[ho : ho + 64, hp, qlo:qhi],
                                start=True, stop=True,
                            )
                            if kc <= 3:  # diag sub-block tq == kc
                                off = 128 * kc - qlo
                                nc.vector.tensor_add(
                                    st[:, off : off + 128],
                                    st[:, off : off + 128],
                                    m_diag[:],
                                )
                            if kc >= 4:  # far sub-block tq == kc-4
                                off = 128 * (kc - 4) - qlo
                                nc.vector.tensor_add(
                                    st[:, off : off + 128],
                                    st[:, off : off + 128],
                                    m_far[:],
                                )
                            nc.scalar.activation(
                                p_sb[:, kc, qlo:qhi], st[:, 0:w],
                                AF.Exp, bias=kv_t[:, kc, :],
                            )
                        for tq in range(TT):
                            ot = po.tile([128, 65], F32, tag="av")
                            for i, kc in enumerate(range(tq, tq + 5)):
                                nc.tensor.matmul(
                                    ot[:],
                                    p_sb[:, kc, 128 * tq : 128 * (tq + 1)],
                                    v_aug[:, kc, 65 * h : 65 * (h + 1)],
                                    start=(i == 0), stop=(i == 4),
                                )
                            rec = small.tile([128, 1], F32, tag="rec")
                            nc.vector.reciprocal(rec[:], ot[:, 64:65])
                            nc.vector.tensor_mul(
                                rec[:], rec[:], gm_t[:, tq, h : h + 1]
                            )
                            nc.scalar.activation(
                                qkv_nat[:, tq, 0, 64 * h : 64 * (h + 1)],
                                ot[:, 0:64], AF.Copy, scale=rec[:],
                            )

                # ---- o transpose + wo + residual ----
                with tc.tile_pool(name=f"ps_wo_{l}", bufs=3, space="PSUM") as pw:
                    for tq in range(TT):
                        for hp in range(FC):
                            tp = pw.tile([128, 128], BF16, tag="tp_o")
                            nc.tensor.transpose(
                                tp[:],
                                qkv_nat[:, tq, 0, 128 * hp : 128 * (hp + 1)],
                                ident[:],
                            )
                            nc.vector.tensor_copy(
                                qT[:, hp, 128 * tq : 128 * (tq + 1)], tp[:]
                            )
                    for mc in range(FC):
                        wos = wopool.tile([128, FC, 128], BF16, tag="wo_s")
                        nc.scalar.dma_start(
                            wos[:], wo[l, mc].rearrange("(kc p) n -> p kc n", p=128)
                        )
                        pr = pw.tile([128, TOK], F32, tag="wo_ps")
                        for kc in range(FC):
                            nc.tensor.matmul(
                                pr[:], wos[:, kc, :], qT[:, kc, :],
                                start=(kc == 0), stop=(kc == FC - 1),
                            )
                        nc.vector.tensor_add(xT[:, mc, :], xT[:, mc, :], pr[:])
                        nc.vector.tensor_copy(xbf[:, mc, :], xT[:, mc, :])

                # ---- FFN ----
                with tc.tile_pool(name=f"ps_ffn_{l}", bufs=2, space="PSUM") as pf:
                    ssq_sb = norm_stats(pf, f"n2_{l}")
                    # combined double-rmsnorm scale on [1, TOK]:
                    # a1 = var+EPS ; t = var/a1 + EPS (=var2+EPS) ; t *= a1
                    # rs = t^-0.5   (extra +EPS inside rsqrt_act is ~6e-8 rel)
                    a1 = rowpool.tile([1, TOK], F32, tag="v2")
                    nc.vector.tensor_scalar(
                        a1[:], ssq_sb[:], 1.0 / DIM, EPS,
                        mybir.AluOpType.mult, mybir.AluOpType.add,
                    )
                    r1 = rowpool.tile([1, TOK], F32, tag="v3")
                    nc.vector.reciprocal(r1[:], a1[:])
                    nc.vector.tensor_scalar_mul(ssq_sb[:], ssq_sb[:], 1.0 / DIM)
                    nc.vector.tensor_mul(ssq_sb[:], ssq_sb[:], r1[:])
                    nc.vector.tensor_scalar_add(ssq_sb[:], ssq_sb[:], EPS)
                    nc.vector.tensor_mul(ssq_sb[:], ssq_sb[:], a1[:])
                    rsqrt_act(r1[:], ssq_sb[:], eps1[:], a1[:], scale=1.0)
                    nc.gpsimd.partition_broadcast(rsb[:], r1[:])
                    for kc in range(FC):
                        nc.vector.tensor_mul(fT[:, kc, :], xT[:, kc, :], rsb[:])

                    nc.scalar.dma_start(
                        b1a_all[:], b1a[l].rearrange("(j p) -> p j", p=128)
                    )
                    nc.scalar.dma_start(
                        b1g_all[:], b1g[l].rearrange("(j p) -> p j", p=128)
                    )
                    nc.scalar.dma_start(
                        b2_all[:], b2[l].rearrange("(j p) -> p j", p=128)
                    )
                    # w1: hidT[j] = gelu-gated product
                    for j in range(HC):
                        pa_ = pf.tile([128, TOK], F32, tag="w1a")
                        pg_ = pf.tile([128, TOK], F32, tag="w1g")
                        wa = w1pool.tile([128, FC, 128], BF16, tag="w1_s")
                        wg_ = w1pool.tile([128, FC, 128], BF16, tag="w1_s")
                        nc.sync.dma_start(
                            wa[:], w1[l, j].rearrange("(kc p) n -> p kc n", p=128)
                        )
                        nc.scalar.dma_start(
                            wg_[:], w1[l, HC + j].rearrange("(kc p) n -> p kc n", p=128)
                        )
                        for kc in range(FC):
                            nc.tensor.matmul(
                                pa_[:], wa[:, kc, :], fT[:, kc, :],
                                start=(kc == 0), stop=(kc == FC - 1),
                            )
                        for kc in range(FC):
                            nc.tensor.matmul(
                                pg_[:], wg_[:, kc, :], fT[:, kc, :],
                                start=(kc == 0), stop=(kc == FC - 1),
                            )
                        gsb = scratch.tile([128, TOK], BF16, tag="gsb")
                        nc.scalar.activation(
                            hidT[:, j, :], pa_[:], AF.Identity, bias=b1a_all[:, j : j + 1]
                        )
                        nc.scalar.activation(gsb[:], pg_[:], AF.Gelu, bias=b1g_all[:, j : j + 1])
                        nc.vector.tensor_mul(hidT[:, j, :], hidT[:, j, :], gsb[:])

                    # w2 + bias + residual
                    for mc in range(FC):
                        w2s = w2pool.tile([128, HC, 128], BF16, tag="w2_s")
                        nc.sync.dma_start(
                            w2s[:], w2[l, mc].rearrange("(kc p) n -> p kc n", p=128)
                        )
                        pr = pf.tile([128, TOK], F32, tag="w2_ps")
                        for kc in range(HC):
                            nc.tensor.matmul(
                                pr[:], w2s[:, kc, :], hidT[:, kc, :],
                                start=(kc == 0), stop=(kc == HC - 1),
                            )
                        fsb = scratch2.tile([128, TOK], F32, tag="fsb")
                        nc.scalar.activation(fsb[:], pr[:], AF.Identity, bias=b2_all[:, mc : mc + 1])
                        nc.vector.tensor_add(xT[:, mc, :], xT[:, mc, :], fsb[:])
                        if l < L - 1:
                            nc.vector.tensor_copy(xbf[:, mc, :], xT[:, mc, :])

            # ---- final rmsnorm + output ----
            with tc.tile_pool(name="ps_fin", bufs=2, space="PSUM") as pfin:
                ssq_sb = norm_stats(pfin, "fin")
                lnf = rowpool.tile([1, TOK], F32, tag="v2")
                rsf = rowpool.tile([1, TOK], F32, tag="v3")
                rsqrt_act(rsf[:], ssq_sb[:], eps1[:], lnf[:], scale=1.0 / DIM)
                nc.gpsimd.partition_broadcast(rsb[:], rsf[:])
                for kc in range(FC):
                    ot = scratch2.tile([128, TOK], F32, tag="onetime")
                    nc.vector.tensor_mul(ot[:], xT[:, kc, :], rsb[:])
                    nc.sync.dma_start(outT[128 * kc : 128 * (kc + 1), :], ot[:])

    nc.compile()
    return nc


_NC_CACHE = None
LAST_RESULT = None


def _get_nc():
    global _NC_CACHE
    if _NC_CACHE is None:
        _NC_CACHE = build_nc()
    return _NC_CACHE


def _prep_weights(inputs):
    """Host-side: permute/pad/cast weights. Returns dict of shared arrays."""
    bf = ml_dtypes.bfloat16
    wq_ = np.asarray(inputs["wq"], np.float32)
    wkv = np.asarray(inputs["wkv"], np.float32)
    wk_, wv_ = wkv[..., : H * DH], wkv[..., H * DH :]
    # deinterleave rope pairs per head: evens then odds
    perm = np.concatenate([np.arange(0, DH, 2), np.arange(1, DH, 2)])
    full_perm = (np.arange(H)[:, None] * DH + perm[None, :]).reshape(-1)
    wq_p = wq_[:, :, full_perm].astype(bf)
    wk_p = wk_[:, :, full_perm].astype(bf)
    wv_b = wv_.astype(bf)
    wgm_b = np.concatenate(
        [np.asarray(inputs["wg"], np.float32), np.asarray(inputs["wmix"], np.float32)],
        axis=-1,
    ).astype(bf)
    wo_ = np.asarray(inputs["wo"], np.float32).astype(bf)  # [L, HD, DIM]
    wo_r = np.ascontiguousarray(
        wo_.reshape(L, H * DH, FC, 128).transpose(0, 2, 1, 3)
    )  # [L, FC, HD, 128]
    w1_ = np.asarray(inputs["w1"], np.float32)
    w1p = np.zeros((L, DIM, 2 * DINP), np.float32)
    w1p[:, :, :DIN] = w1_[:, :, :DIN]
    w1p[:, :, DINP : DINP + DIN] = w1_[:, :, DIN:]
    w1_r = np.ascontiguousarray(
        w1p.astype(bf).reshape(L, DIM, 2 * HC, 128).transpose(0, 2, 1, 3)
    )  # [L, 2*HC, DIM, 128]
    w2_ = np.asarray(inputs["w2"], np.float32)
    w2p = np.zeros((L, DINP, DIM), np.float32)
    w2p[:, :DIN, :] = w2_
    w2_r = np.ascontiguousarray(
        w2p.astype(bf).reshape(L, DINP, FC, 128).transpose(0, 2, 1, 3)
    )  # [L, FC, DINP, 128]
    b1_ = np.asarray(inputs["b1"], np.float32)
    b1a = np.zeros((L, DINP), np.float32)
    b1g = np.zeros((L, DINP), np.float32)
    b1a[:, :DIN] = b1_[:, :DIN]
    b1g[:, :DIN] = b1_[:, DIN:]
    b2_ = np.asarray(inputs["b2"], np.float32)
    return dict(
        wq=wq_p, wk=wk_p, wv=wv_b, wgm=wgm_b, wo=wo_r, w1=w1_r, w2=w2_r,
        b1a=b1a, b1g=b1g, b2=b2_,
    )


def kernel(**inputs):
    nc = _get_nc()
    shared = _prep_weights(inputs)
    x = np.asarray(inputs["x"], np.float32)
    inv = 1.0 / (10000.0 ** (np.arange(0, DH, 2, dtype=np.float32) / DH))
    in_maps = []
    for c in range(N_CORES):
        b, j = c // 4, c % 4
        s0 = TOK * j
        pos = (s0 + np.arange(TOK, dtype=np.float32))[:, None] * inv[None, :]
        kvv = np.zeros(KEYS, np.float32)
        if j == 0:
            kvv[:WIN] = NEG
        m = dict(shared)
        m["xT0"] = np.ascontiguousarray(x[b, s0 : s0 + TOK, :].T)
        m["cos_in"] = np.cos(pos).astype(ml_dtypes.bfloat16)
        m["sin_in"] = np.sin(pos).astype(ml_dtypes.bfloat16)
        m["keyvalid"] = kvv
        in_maps.append(m)
    global LAST_RESULT
    r = run_bass_kernel_spmd(nc, in_maps, core_ids=list(range(N_CORES)))
    LAST_RESULT = r
    out = np.zeros((B, S, DIM), np.float32)
    for c in range(N_CORES):
        b, j = c // 4, c % 4
        out[b, TOK * j : TOK * (j + 1), :] = r.results[c]["outT"].T
    return out
